# revision 15
# baseline (speedup 1.0000x reference)
"""ResNet BasicBlock (conv3x3-BN-ReLU-conv3x3-BN-add-ReLU) on 8 TRN2 NeuronCores.

Data-parallel over batch (4 images per core). Convs are implicit GEMM on the
TensorEngine: 9 shifted-window bf16 matmuls accumulated per PSUM row-tile
(inputs/weights bf16, accumulation and stats fp32). Training-mode BatchNorm
is exact sync-BN: per-core (sum, sumsq) partials go through a tiny AllGather,
every core reduces them and applies the affine locally. A throwaway AllGather
at kernel start absorbs the cross-core collective-init barrier; a dummy Sqrt
preloads the ACT table set. The BN rsqrt runs as ACT Sqrt + one Newton step +
DVE reciprocal (ACT Sqrt alone is low-precision). Final stage fuses
relu(scale*y + bias + x) as one DVE scalar_tensor_tensor + one ACT Relu per
row-chunk, pipelined with the output DMA.
"""

import functools
from contextlib import ExitStack

import ml_dtypes
import numpy as np

from concourse import bacc, bass, mybir, tile
from concourse.bass_utils import run_bass_kernel_spmd

F32 = mybir.dt.float32
F32R = mybir.dt.float32r
BF16 = mybir.dt.bfloat16
AF = mybir.ActivationFunctionType
ALU = mybir.AluOpType

N_CORES = 8
B, C, H, W = 32, 128, 56, 56
B_SH = B // N_CORES           # 4 images per core
HP, WP = H + 2, W + 2         # 58 (zero-padded)
ROWS = 8                      # output rows per conv tile
TPB = H // ROWS               # 7 tiles per image
NT = B_SH * TPB               # 28 tiles per conv per core
N_GLOB = B * H * W            # BN sample count
EPS = 1e-5


def _build():
    nc = bacc.Bacc(
        "TRN2",
        target_bir_lowering=False,
        debug=False,
        enable_asserts=False,
        num_devices=N_CORES,
    )

    xp_d = nc.dram_tensor("xp", [B_SH, C, HP, WP], BF16, kind="ExternalInput")
    w1_d = nc.dram_tensor("w1t", [C, 9 * C], BF16, kind="ExternalInput")
    w2_d = nc.dram_tensor("w2t", [C, 9 * C], BF16, kind="ExternalInput")
    g1_d = nc.dram_tensor("g1", [C, 1], F32, kind="ExternalInput")
    b1_d = nc.dram_tensor("b1", [C, 1], F32, kind="ExternalInput")
    g2_d = nc.dram_tensor("g2", [C, 1], F32, kind="ExternalInput")
    b2_d = nc.dram_tensor("b2", [C, 1], F32, kind="ExternalInput")
    xr_d = nc.dram_tensor("xr", [B_SH, C, H, W], BF16, kind="ExternalInput")
    out_d = nc.dram_tensor("out", [B_SH, C, H, W], F32, kind="ExternalOutput")

    with tile.TileContext(nc) as tc, ExitStack() as ctx:
        const = ctx.enter_context(tc.tile_pool(name="const", bufs=1))
        main = ctx.enter_context(tc.tile_pool(name="main", bufs=1))
        scr = ctx.enter_context(tc.tile_pool(name="scr", bufs=1))
        pp = ctx.enter_context(tc.tile_pool(name="pp", bufs=8, space="PSUM"))
        dram = ctx.enter_context(tc.tile_pool(name="dram", bufs=1, space="DRAM"))

        # --- collective warm-up -------------------------------------------
        # The first collective pays the cross-core init barrier plus mesh
        # setup (~30-40us observed). Fire a tiny throwaway AllGather first
        # so it absorbs that cost while the input DMAs / conv1 run.
        warm_in = dram.tile([8, 32], F32, name="warm_in", tag="warm_in")
        warm_out = dram.tile(
            [N_CORES, 8, 32], F32, name="warm_out", tag="warm_out",
            addr_space="Shared",
        )
        nc.gpsimd.collective_compute(
            "AllGather",
            ALU.bypass,
            ins=[warm_in[:].opt()],
            outs=[warm_out[:].opt()],
            replica_groups=[list(range(N_CORES))],
        )

        # --- params + input, in critical-path order -----------------------
        # Chain the big DMAs so conv1's first tiles (w1 + x image 0) land
        # first instead of all transfers sharing bandwidth concurrently.
        from concourse.bass import _add_dep_helper

        xp_sb = []
        prev = None
        for b in range(B_SH):
            t = main.tile([C, HP, WP], BF16, name=f"xp{b}", tag=f"xp{b}")
            if b == 0:
                # split image 0 so conv1's first row-tiles unblock early
                d = nc.scalar.dma_start(t[:, 0:29, :], xp_d[b][:, 0:29, :])
                d2 = nc.scalar.dma_start(t[:, 29:, :], xp_d[b][:, 29:, :])
                _add_dep_helper(d2.ins, d.ins, sync=True, reason="dma priority chain")
                prev = d2
            else:
                d = nc.scalar.dma_start(t[:], xp_d[b])
                _add_dep_helper(d.ins, prev.ins, sync=True, reason="dma priority chain")
                prev = d
            xp_sb.append(t)

        w1_sb = const.tile([C, 9 * C], BF16, name="w1_sb", tag="w1_sb")
        nc.sync.dma_start(w1_sb[:], w1_d[:])
        w2_sb = const.tile([C, 9 * C], BF16, name="w2_sb", tag="w2_sb")
        bn_par = {}
        for nm in ("g1", "b1", "g2", "b2"):
            bn_par[nm] = const.tile([C, 1], F32, name=f"{nm}_sb", tag=f"{nm}_sb")

        y1p = []  # conv1 raw output, padded buffer (later normalized in place)
        for b in range(B_SH):
            t = main.tile([C, HP, WP], BF16, name=f"y1p{b}", tag=f"y1p{b}")
            # zero the 1-px frame (interior is fully overwritten by conv1)
            nc.gpsimd.memset(t[:, 0, :], 0.0)
            nc.gpsimd.memset(t[:, HP - 1, :], 0.0)
            nc.gpsimd.memset(t[:, :, 0], 0.0)
            nc.gpsimd.memset(t[:, :, WP - 1], 0.0)
            y1p.append(t)

        y2 = []
        out_sb = []
        xin_sb = []
        for b in range(B_SH):
            t = main.tile([C, H, W], BF16, name=f"y2_{b}", tag=f"y2_{b}")
            y2.append(t)
            t2 = main.tile([C, H, W], F32, name=f"os_{b}", tag=f"os_{b}")
            out_sb.append(t2)
            t3 = main.tile([C, H, W], BF16, name=f"xr_{b}", tag=f"xr_{b}")
            xin_sb.append(t3)

        # prewarm the ACT sqrt table set (Copy/Relu ride along in every set)
        warm_act = scr.tile([C, 1], F32, name="warm_act", tag="warm_act")
        nc.vector.memset(warm_act[:], 1.0)
        nc.scalar.activation(warm_act[:], warm_act[:], AF.Sqrt)

        # per-tile BN partials, one column per conv tile
        st = {}
        for nm in ("s1", "q1", "s2", "q2"):
            st[nm] = scr.tile([C, NT], F32, name=f"st_{nm}", tag=f"st_{nm}")

        sq_scr = scr.tile([C, ROWS, W], F32, name="sq_scr", tag="sq_scr")

        def conv(x_tiles, w_sb, writer):
            for b in range(B_SH):
                for t in range(TPB):
                    idx = b * TPB + t
                    h0 = t * ROWS
                    ps = pp.tile([C, ROWS, W], F32, name="ps", tag="ps")
                    for ky in range(3):
                        for kx in range(3):
                            tap = ky * 3 + kx
                            rhs = x_tiles[b][:, h0 + ky : h0 + ky + ROWS, kx : kx + W]
                            nc.tensor.matmul(
                                ps[:],
                                w_sb[:, tap * C : (tap + 1) * C],
                                rhs,
                                start=(tap == 0),
                                stop=(tap == 8),
                            )
                    writer(b, t, idx, ps)

        def stat_writer(dst_of, s_tile, q_tile):
            def w(b, t, idx, ps):
                # PSUM -> SBUF drain + per-channel sum on ScalarE
                dst = dst_of(b, t)
                nc.scalar.activation(
                    dst, ps[:], AF.Copy, accum_out=s_tile[:, idx : idx + 1]
                )
                # sum of squares on VectorE, from the SBUF copy (PSUM has
                # only one DVE read port; tensor_tensor_reduce faults on hw)
                src = dst
                nc.vector.scalar_tensor_tensor(
                    sq_scr[:],
                    src,
                    1.0,
                    src,
                    ALU.mult,
                    ALU.mult,
                    accum_out=q_tile[:, idx : idx + 1],
                )

            return w

        def sync_stats(s_tile, q_tile, tag):
            loc = scr.tile([C, 2], F32, name=f"loc{tag}", tag=f"loc{tag}")
            nc.vector.tensor_reduce(loc[:, 0:1], s_tile[:], mybir.AxisListType.X, ALU.add)
            nc.vector.tensor_reduce(loc[:, 1:2], q_tile[:], mybir.AxisListType.X, ALU.add)
            cc_in = dram.tile([C, 2], F32, name=f"ccin{tag}", tag=f"ccin{tag}")
            cc_out = dram.tile(
                [N_CORES, C, 2], F32, name=f"ccout{tag}", tag=f"ccout{tag}",
                addr_space="Shared",
            )
            nc.sync.dma_start(cc_in[:], loc[:])
            nc.gpsimd.collective_compute(
                "AllGather",
                ALU.bypass,
                ins=[cc_in[:].opt()],
                outs=[cc_out[:].opt()],
                replica_groups=[list(range(N_CORES))],
            )
            graw = scr.tile([C, N_CORES, 2], F32, name=f"graw{tag}", tag=f"graw{tag}")
            nc.sync.dma_start(graw[:], cc_out[:].transpose([1, 0, 2]))
            glob = scr.tile([C, 2], F32, name=f"glob{tag}", tag=f"glob{tag}")
            nc.vector.tensor_reduce(
                glob[:], graw[:].transpose([0, 2, 1]), mybir.AxisListType.X, ALU.add
            )
            return glob

        def bn_coef(glob, g_sb, b_sb, tag):
            cf = scr.tile([C, 16], F32, name=f"cf{tag}", tag=f"cf{tag}")
            col = lambda i: cf[:, i : i + 1]
            negm, veps, s0, r0, tnw, s1, inv, scl, nscl, bia = (
                col(i) for i in range(2, 12)
            )
            me = cf[:, 0:2]  # [mean, ex2]
            mean, ex2 = cf[:, 0:1], cf[:, 1:2]
            nc.vector.tensor_scalar_mul(me, glob[:], 1.0 / N_GLOB)
            nc.vector.tensor_scalar_mul(negm, mean, -1.0)
            # veps = ex2 - mean^2 + EPS
            nc.vector.scalar_tensor_tensor(veps, mean, negm, ex2, ALU.mult, ALU.add)
            nc.vector.tensor_scalar_add(veps, veps, EPS)
            # rsqrt(veps): ACT sqrt (low precision) + one Newton step, then
            # exact-ish DVE reciprocal.
            nc.scalar.activation(s0, veps, AF.Sqrt)
            nc.vector.reciprocal(r0, s0)
            nc.vector.scalar_tensor_tensor(tnw, veps, r0, s0, ALU.mult, ALU.add)
            nc.vector.tensor_scalar_mul(s1, tnw, 0.5)
            nc.vector.reciprocal(inv, s1)
            nc.vector.tensor_scalar(scl, inv, g_sb[:], None, ALU.mult)
            nc.vector.tensor_scalar_mul(nscl, scl, -1.0)
            # bias = beta - mean * scale
            nc.vector.scalar_tensor_tensor(bia, mean, nscl, b_sb[:], ALU.mult, ALU.add)
            return scl, bia

        # ============ conv1 + BN1 stats ============
        conv(
            xp_sb,
            w1_sb,
            stat_writer(
                lambda b, t: y1p[b][:, 1 + t * ROWS : 1 + (t + 1) * ROWS, 1 : 1 + W],
                st["s1"],
                st["q1"],
            ),
        )
        # deferred: conv2 weights, BN params, dense x for the residual read
        nc.sync.dma_start(w2_sb[:], w2_d[:])
        for b in range(B_SH):
            nc.sync.dma_start(xin_sb[b][:], xr_d[b])
        for nm, dram_t in (("g1", g1_d), ("b1", b1_d), ("g2", g2_d), ("b2", b2_d)):
            nc.sync.dma_start(bn_par[nm][:], dram_t[:])

        glob1 = sync_stats(st["s1"], st["q1"], "1")
        scl1, bia1 = bn_coef(glob1, bn_par["g1"], bn_par["b1"], "1")

        # normalize + relu, in place (interior only; border stays zero).
        # image 0 is split so conv2's first row-tiles unblock asap.
        norm_chunks = [(0, 0, 10), (0, 10, 24), (0, 34, 22), (1, 0, 56), (2, 0, 56), (3, 0, 56)]
        for b, r0, nr in norm_chunks:
            itr = y1p[b][:, 1 + r0 : 1 + r0 + nr, 1 : 1 + W]
            nc.scalar.activation(itr, itr, AF.Relu, bias=bia1, scale=scl1)

        # ============ conv2 + BN2 stats ============
        conv(
            y1p,
            w2_sb,
            stat_writer(
                lambda b, t: y2[b][:, t * ROWS : (t + 1) * ROWS, :],
                st["s2"],
                st["q2"],
            ),
        )
        glob2 = sync_stats(st["s2"], st["q2"], "2")
        scl2, bia2 = bn_coef(glob2, bn_par["g2"], bn_par["b2"], "2")

        # ============ final: relu(y2*scl2 + bia2 + x) ============
        # chunked so DVE / ACT / DMA-out pipeline against each other
        FH = H // 4
        for b in range(B_SH):
            for quarter in range(4):
                r0 = quarter * FH
                ys = y2[b][:, r0 : r0 + FH, :]
                xs = xin_sb[b][:, r0 : r0 + FH, :]
                os = out_sb[b][:, r0 : r0 + FH, :]
                nc.vector.scalar_tensor_tensor(ys, ys, scl2, xs, ALU.mult, ALU.add)
                nc.scalar.activation(os, ys, AF.Relu, bias=bia2, scale=1.0)
                nc.sync.dma_start(out_d[b][:, r0 : r0 + FH, :], os)

    return nc


@functools.lru_cache(maxsize=1)
def get_nc():
    nc = _build()
    nc.compile()
    return nc


def make_in_maps(x, w1, gamma1, beta1, w2, gamma2, beta2):
    x = np.ascontiguousarray(np.asarray(x, dtype=np.float32))
    xp = np.zeros((B, C, HP, WP), ml_dtypes.bfloat16)
    xp[:, :, 1 : 1 + H, 1 : 1 + W] = x.astype(ml_dtypes.bfloat16)
    # w[o,i,ky,kx] -> [i, (ky,kx,o)] so tap t's lhsT slice is [C_in, C_out]
    w1t = np.ascontiguousarray(
        np.asarray(w1, np.float32).transpose(1, 2, 3, 0)
    ).reshape(C, 9 * C).astype(ml_dtypes.bfloat16)
    w2t = np.ascontiguousarray(
        np.asarray(w2, np.float32).transpose(1, 2, 3, 0)
    ).reshape(C, 9 * C).astype(ml_dtypes.bfloat16)
    g1 = np.ascontiguousarray(np.asarray(gamma1, np.float32).reshape(C, 1))
    b1 = np.ascontiguousarray(np.asarray(beta1, np.float32).reshape(C, 1))
    g2 = np.ascontiguousarray(np.asarray(gamma2, np.float32).reshape(C, 1))
    b2 = np.ascontiguousarray(np.asarray(beta2, np.float32).reshape(C, 1))
    xr = x.astype(ml_dtypes.bfloat16)
    maps = []
    for i in range(N_CORES):
        maps.append(
            {
                "xp": np.ascontiguousarray(xp[i * B_SH : (i + 1) * B_SH]),
                "xr": np.ascontiguousarray(xr[i * B_SH : (i + 1) * B_SH]),
                "w1t": w1t,
                "w2t": w2t,
                "g1": g1,
                "b1": b1,
                "g2": g2,
                "b2": b2,
            }
        )
    return maps


def run(in_maps, trace=False, **kwargs):
    nc = get_nc()
    return run_bass_kernel_spmd(
        nc, in_maps, core_ids=list(range(N_CORES)), trace=trace, **kwargs
    )


def kernel(x, w1, gamma1, beta1, w2, gamma2, beta2):
    maps = make_in_maps(x, w1, gamma1, beta1, w2, gamma2, beta2)
    res = run(maps)
    out = np.concatenate([res.results[i]["out"] for i in range(N_CORES)], axis=0)
    return np.ascontiguousarray(out.astype(np.float32))
